# revision 42
# baseline (speedup 1.0000x reference)
"""Trainium2 Bass kernel for nn_LSTMHarmonizer — V3 (transfer-optimized).

The on-device kernel (per core: gx GEMM -> 1024-step LSTM scan -> 3-head
GEMM) runs in ~1ms; the axon tunnel to the devices moves ~40-60 MB/s, so
wall time is dominated by host<->device bytes.  V3 therefore:
  - caches the jitted shard_map executable across kernel() calls (the
    stock run_bass_kernel_spmd re-traces + re-lowers every call),
  - keeps weights device-resident (uploaded once, content-checked),
  - emits logits in f16 (50 MB down instead of 100), upcast on host,
  - donates the PREVIOUS call's device-resident output as the next
    call's output seed (kernel writes every element, so no 100 MB zeros
    upload per call),
  - fetches output shards with a small thread pool (~1.4x).

Scan math (unchanged from V2): gate tiles m = f0 g0 i0 o0 f1 g1 i1 o1;
c' tracks 2c with g rows pre-scaled 2x (tanh via 2*sigma(2x)-1); h is
stored as h/2 in bf16 with W_hh and head_w pre-scaled 2x.
"""

import contextlib
import zlib
from concurrent.futures import ThreadPoolExecutor

import numpy as np
import ml_dtypes

import jax
import jax.numpy as jnp
from jax.sharding import Mesh, PartitionSpec, NamedSharding
from jax.experimental.shard_map import shard_map

import concourse.bass as bass
import concourse.mybir as mybir
from concourse import bass2jax

BF16 = ml_dtypes.bfloat16

B, T, D, H, V, NV = 64, 1024, 128, 256, 128, 3
G4 = 4 * H            # 1024
NC = 8                # cores
BC = B // NC          # 8 sequences per core
NVV = NV * V          # 384
TOK = BC * T          # 8192 tokens per core

# k1-phase / o-phase tile orders (m indices)
CJ = [0, 1, 2, 3, 4, 5]       # c-gates: f0 g0 i0, f1 g1 i1
OG = [(6, 0), (7, 0), (6, 1), (7, 1)]  # o-gates (m, k)


def build_nc(Tc=T):
    TOKc = BC * Tc
    NT3 = TOKc // 128          # phase-3 token tiles
    NCH = TOKc // 512          # phase-1 token chunks
    f32 = mybir.dt.float32
    bf16 = mybir.dt.bfloat16
    f16 = mybir.dt.float16

    i8 = mybir.dt.int8

    nc = bass.Bass()
    xT_d = nc.declare_dram_parameter("xT", [128, TOKc], bf16, isOutput=False)
    wih_d = nc.declare_dram_parameter("wihT", [128, G4], bf16, isOutput=False)
    whh_d = nc.declare_dram_parameter("whhT", [128, 16 * 128], bf16, isOutput=False)
    hw_d = nc.declare_dram_parameter("headwT", [128, 2 * NVV], bf16, isOutput=False)
    bias_d = nc.declare_dram_parameter("biasm", [128, 8], f32, isOutput=False)
    hb_d = nc.declare_dram_parameter("headb", [128, NVV], f32, isOutput=False)
    id_d = nc.declare_dram_parameter("ident", [128, 128], bf16, isOutput=False)
    z_d = nc.declare_dram_parameter("zeros16", [128, 16], f32, isOutput=False)
    cst_d = nc.declare_dram_parameter("consts", [128, 4], f32, isOutput=False)
    lg_d = nc.declare_dram_parameter("logits", [NV, BC, Tc, V], i8, isOutput=True)
    # last column carries the per-call nonce (stale-readback canary)
    sc_d = nc.declare_dram_parameter("scales", [128, TOKc // 128 + 2], f32,
                                     isOutput=True)

    ctx = contextlib.ExitStack()
    with ctx:
        sb = lambda name, shape, dt: ctx.enter_context(
            nc.sbuf_tensor(name, shape, dt))
        ps = lambda name, shape: ctx.enter_context(
            nc.psum_tensor(name, shape, f32))
        sem = lambda name: ctx.enter_context(nc.semaphore(name))

        xT = sb("xT_s", [128, TOKc], bf16)
        wih = sb("wih_s", [128, G4], bf16)
        whh = sb("whh_s", [128, 16 * 128], bf16)
        hw = sb("hw_s", [128, 2 * NVV], bf16)
        biasm = sb("biasm_s", [128, 8], f32)
        headb = sb("headb_s", [128, NVV], f32)
        ident = sb("ident_s", [128, 128], bf16)
        zeros16 = sb("zeros16_s", [128, 16], f32)
        cst = sb("cst_s", [128, 4], f32)   # 0.5, 4.0, 2^23, nonce
        gx = sb("gx_s", [128, 8 * TOKc], bf16)      # (t, m, b)
        hh = sb("hh_s", [128, 2 * TOKc], bf16)      # (j, t, b), holds h/2
        sall = sb("sall_s", [128, 64], f32)        # sigma of gates (m, b)
        scb = sb("scb_s", [128, 16], f32)          # sigma(c') per (j, b)
        vu = sb("vu_s", [128, 32], f32)            # v (0:16), u (16:32)
        cps = sb("cps_s", [128, 16], f32)          # c' (=2c): j0 0:8, j1 8:16
        # phase-3 int8 quantization buffers (partition = token-in-tile)
        scl = sb("scl_s", [128, NT3], f32)         # per-token absmax
        sclm = sb("sclm_s", [128, NT3], f32)       # clamped absmax
        sinv = sb("sinv_s", [128, NT3], f32)       # 1/absmax
        sqs = sb("sqs_s", [128, NT3 + 2], f32)     # 126/absmax|nonce|xsum
        tmpb = sb("tmpb_s", [128, 2 * NVV], f32)   # s*x + 2^23, 2 slots
        qout = sb("qout_s", [128, 4 * NVV], i8)    # int8 evict slots

        # 6 full banks: [0..3] phases 1/3; scan uses [t%2], [2+t%2], [4+t%2]
        ps_big = [ps(f"psb{i}", [128, 512]) for i in range(6)]

        dma_in = sem("dma_in")
        mm1 = sem("mm1")
        ev1a = sem("ev1a")
        ev1d = sem("ev1d")
        s_mm = sem("s_mm")     # 3/step: c-j0, c-j1, o
        s_sig = sem("s_sig")   # 5/step: A1 A3 A2' sc0 sc1
        s_cp = sem("s_cp")     # init=2, then 2/step: c'0 (2t+3), c'1 (2t+4)
        s_vu = sem("s_vu")     # 2/step: u0 (2t+1), u1 (2t+2)
        s_h = sem("s_h")       # 2/step: h0, h1
        mm3 = sem("mm3")
        s_pa = sem("s_pa")     # 1/tile: pass A (psum consumed)
        s_sc = sem("s_sc")     # 1: applied scales ready
        s_t2 = sem("s_t2")     # 1/tile: pass B scalar (s*x + 2^23)
        ev3 = sem("ev3")       # 1/tile: pass B vector (int8 slot ready)
        dma_out = sem("dma_out")

        ALU = mybir.AluOpType
        AF = mybir.ActivationFunctionType

        # Semaphores are NOT cleared on allocation, and nothing clears them
        # between executions of a loaded NEFF (the target_bir_lowering
        # preamble that would is not emitted on the bass_exec path).  On
        # re-execution every wait_ge would pass against stale counts and all
        # engines would free-run.  Clear our sems and barrier before use.
        all_sems = [dma_in, mm1, ev1a, ev1d, s_mm, s_sig, s_cp, s_vu, s_h,
                    mm3, s_pa, s_sc, s_t2, ev3, dma_out]
        for rng in bass.compact_to_ranges([s.num for s in all_sems]):
            nc.gpsimd.dma_reset(rng)
            nc.gpsimd.sem_clear(rng)
        nc.all_engine_barrier()

        gx_v = gx[:].rearrange("p (t m b) -> p t m b", t=Tc, m=8, b=BC)

        def gx_evict_view(m, ch):
            return gx_v[:, ch * 64:(ch + 1) * 64, m, :]

        def hh_ap(j, t):
            off = j * TOKc + t * BC
            return hh[:, off:off + BC]

        def bc0(t):
            return ps_big[t % 2]
        def bc1(t):
            return ps_big[2 + t % 2]
        def bo(t):
            return ps_big[4 + t % 2]

        def wt(k, m):
            return whh[:, (k * 8 + m) * 128:(k * 8 + m + 1) * 128]

        with nc.Block() as block:

            @block.sync
            def _(sync):
                sync.dma_start(out=xT[:], in_=xT_d[:]).then_inc(dma_in, 16)
                sync.dma_start(out=wih[:], in_=wih_d[:]).then_inc(dma_in, 16)
                sync.dma_start(out=whh[:], in_=whh_d[:]).then_inc(dma_in, 16)
                sync.dma_start(out=hw[:], in_=hw_d[:]).then_inc(dma_in, 16)
                sync.dma_start(out=biasm[:], in_=bias_d[:]).then_inc(dma_in, 16)
                sync.dma_start(out=headb[:], in_=hb_d[:]).then_inc(dma_in, 16)
                sync.dma_start(out=ident[:], in_=id_d[:]).then_inc(dma_in, 16)
                sync.dma_start(out=zeros16[:], in_=z_d[:]).then_inc(dma_in, 16)
                sync.dma_start(out=cst[:], in_=cst_d[:]).then_inc(dma_in, 16)
                # phase 3 output DMAs
                for tk in range(NT3):
                    sync.wait_ge(ev3, tk + 1)
                    for n in range(NV):
                        dview = lg_d[n, :, tk * 16:(tk + 1) * 16, :].rearrange(
                            "b t v -> t b v")
                        slot = qout[:, (tk % 4) * NVV + n * V:
                                    (tk % 4) * NVV + (n + 1) * V]
                        sync.dma_start(out=dview, in_=slot).then_inc(dma_out, 16)
                sync.wait_ge(s_sc, 1)
                sync.dma_start(out=sc_d[:], in_=sqs[:]).then_inc(dma_out, 16)
                sync.wait_ge(dma_out, 48 * NT3 + 16)

            @block.tensor
            def _(tensor):
                tensor.wait_ge(dma_in, 144)
                # ---- phase 1: gx GEMM ----
                for m in range(8):
                    for ch in range(NCH):
                        idx = m * NCH + ch
                        if idx >= 4:
                            j = idx - 4
                            if j % 2 == 0:
                                tensor.wait_ge(ev1a, j // 2 + 1)
                            else:
                                tensor.wait_ge(ev1d, (j + 1) // 2)
                        tensor.matmul(
                            ps_big[idx % 4][:, :512],
                            lhsT=wih[:, m * 128:(m + 1) * 128],
                            rhs=xT[:, ch * 512:(ch + 1) * 512],
                            start=True, stop=True,
                        ).then_inc(mm1, 1)
                # ---- phase 2: scan ----
                tensor.wait_ge(ev1a, 4 * NCH)
                tensor.wait_ge(ev1d, 4 * NCH)
                def gout(t, m):
                    # psum slot for gate tile m of step t
                    if m < 3:
                        return bc0(t)[:, m * 8:(m + 1) * 8]
                    if m < 6:
                        return bc1(t)[:, (m - 3) * 8:(m - 2) * 8]
                    return bo(t)[:, (m - 6) * 8:(m - 5) * 8]

                gxs = lambda t, a, b2: gx[:, t * 64 + a:t * 64 + b2]
                for t in range(Tc):
                    if t >= 2:
                        tensor.wait_ge(s_sig, 5 * (t - 2) + 3)
                    tensor.matmul(
                        bc0(t)[:, 0:24], lhsT=ident[:], rhs=gxs(t, 0, 24),
                        start=True, stop=(t == 0), skip_group_check=True)
                    tensor.matmul(
                        bc1(t)[:, 0:24], lhsT=ident[:], rhs=gxs(t, 24, 48),
                        start=True, stop=(t == 0), skip_group_check=True)
                    ins0 = tensor.matmul(
                        bo(t)[:, 0:16], lhsT=ident[:], rhs=gxs(t, 48, 64),
                        start=True, stop=(t == 0), skip_group_check=True)
                    if t == 0:
                        ins0.then_inc(s_mm, 1)
                        continue
                    # c-gates k0 phase
                    tensor.wait_ge(s_h, 2 * t - 1)
                    for m in CJ:
                        tensor.matmul(
                            gout(t, m), lhsT=wt(0, m),
                            rhs=hh_ap(0, t - 1), start=False, stop=False,
                            skip_group_check=True)
                    # c-gates k1 phase
                    tensor.wait_ge(s_h, 2 * t)
                    for i, m in enumerate(CJ):
                        ins = tensor.matmul(
                            gout(t, m), lhsT=wt(1, m),
                            rhs=hh_ap(1, t - 1), start=False, stop=(i in (2, 5)),
                            skip_group_check=True)
                        if i == 2:
                            ins.then_inc(s_mm, 1)     # c-j0 done: 3t-1
                        elif i == 5:
                            ins.then_inc(s_mm, 1)     # c-j1 done: 3t
                    # o-gates
                    for i, (m, k) in enumerate(OG):
                        ins = tensor.matmul(
                            gout(t, m), lhsT=wt(k, m),
                            rhs=hh_ap(k, t - 1), start=False,
                            stop=(i == 3), skip_group_check=True)
                        if i == 3:
                            ins.then_inc(s_mm, 1)     # o done: 3t+1
                # ---- phase 3: heads ----
                tensor.wait_ge(s_h, 2 * Tc)
                for tk in range(NT3):
                    if tk >= 4:
                        tensor.wait_ge(s_pa, tk - 3)
                    tensor.matmul(
                        ps_big[tk % 4][:, :NVV],
                        lhsT=hh[:, tk * 128:tk * 128 + 128],
                        rhs=hw[:, :NVV], start=True, stop=False,
                        skip_group_check=True,
                    )
                    tensor.matmul(
                        ps_big[tk % 4][:, :NVV],
                        lhsT=hh[:, TOKc + tk * 128:TOKc + tk * 128 + 128],
                        rhs=hw[:, NVV:2 * NVV], start=False, stop=True,
                        skip_group_check=True,
                    ).then_inc(mm3, 1)

            @block.scalar
            def _(scalar):
                scalar.wait_ge(dma_in, 144)
                # phase-1 evicts: even tiles
                for idx in range(0, 8 * NCH, 2):
                    m, ch = idx // NCH, idx % NCH
                    scalar.wait_ge(mm1, idx + 1)
                    scalar.activation(
                        out=gx_evict_view(m, ch),
                        in_=ps_big[idx % 4][:, :512].rearrange(
                            "p (t b) -> p t b", t=64, b=BC),
                        func=AF.Identity, bias=biasm[:, m:m + 1],
                    ).then_inc(ev1a, 1)
                # scan
                for t in range(Tc):
                    scalar.wait_ge(s_mm, 3 * t - 1 if t else 1)
                    scalar.activation(out=sall[:, 0:24], in_=bc0(t)[:, 0:24],
                                      func=AF.Sigmoid).then_inc(s_sig, 1)
                    scalar.wait_ge(s_mm, 3 * t if t else 1)
                    scalar.activation(out=sall[:, 24:48], in_=bc1(t)[:, 0:24],
                                      func=AF.Sigmoid).then_inc(s_sig, 1)
                    scalar.wait_ge(s_mm, 3 * t + 1)
                    scalar.activation(out=sall[:, 48:64], in_=bo(t)[:, 0:16],
                                      func=AF.Sigmoid).then_inc(s_sig, 1)
                    scalar.wait_ge(s_cp, 2 * t + 2)
                    scalar.activation(out=scb[:, 0:8], in_=cps[:, 0:8],
                                      func=AF.Sigmoid).then_inc(s_sig, 1)
                    scalar.wait_ge(s_cp, 2 * t + 3)
                    scalar.activation(out=scb[:, 8:16], in_=cps[:, 8:16],
                                      func=AF.Sigmoid).then_inc(s_sig, 1)
                # phase-3 pass B (1/2): tmp = s*x + 2^23  (rounds to int grid)
                scalar.wait_ge(s_sc, 1)
                for tk in range(NT3):
                    if tk >= 2:
                        scalar.wait_ge(ev3, tk - 1)
                    scalar.activation(
                        out=tmpb[:, (tk % 2) * NVV:(tk % 2 + 1) * NVV],
                        in_=gx[:, tk * NVV:(tk + 1) * NVV],
                        func=AF.Identity,
                        scale=sqs[:, tk:tk + 1], bias=cst[:, 2:3],
                    ).then_inc(s_t2, 1)

            @block.vector
            def _(vector):
                vector.wait_ge(dma_in, 144)
                # x-upload checksum: per-partition sum of xT (host verifies)
                vector.tensor_reduce(
                    out=sqs[:, NT3 + 1:NT3 + 2], in_=xT[:],
                    axis=mybir.AxisListType.X, op=ALU.add)
                # phase-1 evicts: odd tiles
                for idx in range(1, 8 * NCH, 2):
                    m, ch = idx // NCH, idx % NCH
                    vector.wait_ge(mm1, idx + 1)
                    vector.tensor_scalar_add(
                        gx_evict_view(m, ch),
                        ps_big[idx % 4][:, :512].rearrange(
                            "p (t b) -> p t b", t=64, b=BC),
                        biasm[:, m:m + 1],
                    ).then_inc(ev1d, 1)
                # init c' = 0
                vector.tensor_copy(cps[:], zeros16[:]).then_inc(s_cp, 1)
                # scan: sall cols (m,b): f0 g0 i0 o0 f1 g1 i1 o1
                for t in range(Tc):
                    for j in range(2):
                        o = 24 * j
                        vector.wait_ge(s_sig, 5 * t + 1 + j)
                        vector.wait_ge(s_cp, max(1, 2 * t + j))
                        # v_j = sig(f_j) * c'_old_j
                        vector.tensor_tensor(
                            out=vu[:, j * 8:j * 8 + 8], in0=sall[:, o:o + 8],
                            in1=cps[:, j * 8:(j + 1) * 8], op=ALU.mult)
                        # u_j = (sig(2g_j) - 0.5) * sig(i_j)
                        vector.scalar_tensor_tensor(
                            out=vu[:, 16 + j * 8:24 + j * 8],
                            in0=sall[:, o + 8:o + 16], scalar=cst[:, 0:1],
                            in1=sall[:, o + 16:o + 24],
                            op0=ALU.subtract, op1=ALU.mult).then_inc(s_vu, 1)
                    for j in range(2):
                        # c'_j = 4*u_j + v_j
                        vector.wait_ge(s_vu, 2 * t + 1 + j)
                        vector.scalar_tensor_tensor(
                            out=cps[:, j * 8:(j + 1) * 8],
                            in0=vu[:, 16 + j * 8:24 + j * 8], scalar=cst[:, 1:2],
                            in1=vu[:, j * 8:j * 8 + 8],
                            op0=ALU.mult, op1=ALU.add).then_inc(s_cp, 1)
                    for j in range(2):
                        # h_j/2 = (sc_j - 0.5) * sig(o_j)
                        vector.wait_ge(s_sig, 5 * t + 4 + j)
                        vector.scalar_tensor_tensor(
                            out=hh_ap(j, t), in0=scb[:, j * 8:(j + 1) * 8],
                            scalar=cst[:, 0:1],
                            in1=sall[:, 48 + 8 * j:56 + 8 * j],
                            op0=ALU.subtract, op1=ALU.mult).then_inc(s_h, 1)
                # phase-3 pass A: bias add (bf16, into dead gx region) +
                # per-token absmax of the stored values.  The reduce runs one
                # tile BEHIND the add: a same-engine read immediately after a
                # large SBUF write has been observed to catch stale bytes
                # (write-drain hazard), so every RAW pair below is separated
                # by at least one intervening instruction.
                def lgb(tk):
                    return gx[:, tk * NVV:(tk + 1) * NVV]

                for tk in range(NT3):
                    vector.wait_ge(mm3, tk + 1)
                    vector.tensor_tensor(
                        out=lgb(tk), in0=ps_big[tk % 4][:, :NVV], in1=headb[:],
                        op=ALU.add).then_inc(s_pa, 1)
                    if tk >= 1:
                        vector.tensor_reduce(
                            out=scl[:, tk - 1:tk], in_=lgb(tk - 1),
                            axis=mybir.AxisListType.X, op=ALU.max,
                            apply_absolute_value=True)
                vector.tensor_copy(sqs[:, NT3:NT3 + 1], cst[:, 3:4])  # nonce
                vector.tensor_reduce(
                    out=scl[:, NT3 - 1:NT3], in_=lgb(NT3 - 1),
                    axis=mybir.AxisListType.X, op=ALU.max,
                    apply_absolute_value=True)
                # applied scale s = 126/max(|x|, eps); shipped verbatim.
                # memsets of dead buffers space the RAW chain.
                vector.memset(vu[:, 0:8], 0.0)
                vector.tensor_scalar_max(sclm[:], scl[:], 1e-20)
                vector.memset(vu[:, 8:16], 0.0)
                vector.reciprocal(sinv[:], sclm[:])
                vector.memset(vu[:, 16:24], 0.0)
                vector.tensor_scalar_mul(sqs[:, :NT3], sinv[:], 126.0)
                vector.memset(vu[:, 24:32], 0.0)
                vector.tensor_copy(scb[:, 0:8], zeros16[:, 0:8]).then_inc(
                    s_sc, 1)
                # phase-3 pass B (2/2): int8 = (tmp - 2^23), exact integer
                for tk in range(NT3):
                    vector.wait_ge(s_t2, tk + 1)
                    if tk >= 4:
                        vector.wait_ge(dma_out, 48 * (tk - 3))
                    vector.tensor_scalar(
                        out=qout[:, (tk % 4) * NVV:(tk % 4 + 1) * NVV],
                        in0=tmpb[:, (tk % 2) * NVV:(tk % 2 + 1) * NVV],
                        scalar1=cst[:, 2:3], scalar2=None,
                        op0=ALU.subtract).then_inc(ev3, 1)

    return nc


def _to_bf16(a):
    """Fast f32 -> bf16 with round-to-nearest-even (finite inputs)."""
    u = np.ascontiguousarray(a, np.float32).view(np.uint32)
    r = ((u >> 16) & 1) + np.uint32(0x7FFF)
    return ((u + r) >> 16).astype(np.uint16).view(BF16)


def _prep_weights(W_ih, W_hh, b_ih, b_hh, head_w, head_b):
    # gate order (i,f,g,o) -> m-tiles (f0 g0 i0 o0 f1 g1 i1 o1)
    a = np.arange
    perm = np.concatenate([
        a(256, 384), a(512, 640), a(0, 128),
        a(384, 512), a(640, 768), a(128, 256),
        a(768, 896), a(896, 1024)])
    g_rows = np.concatenate([a(128, 256), a(512, 640)])  # g0, g1 in new order
    wi = W_ih[perm].astype(np.float64).copy()
    wh = W_hh[perm].astype(np.float64).copy()
    bb = (b_ih + b_hh)[perm].astype(np.float64).copy()
    # tanh fold: g rows x2 everywhere; h stored as h/2: W_hh x2, head_w x2
    wi[g_rows] *= 2.0
    bb[g_rows] *= 2.0
    wh *= 2.0
    wh[g_rows] *= 2.0
    hwn = 2.0 * head_w.astype(np.float64)

    wihT = wi.T.astype(BF16)                       # [D, G4]
    whhT_f = wh.T                                  # [H, G4]
    whh_tiles = np.zeros((128, 16 * 128), np.float64)
    for k in range(2):
        for m in range(8):
            whh_tiles[:, (k * 8 + m) * 128:(k * 8 + m + 1) * 128] = \
                whhT_f[k * 128:(k + 1) * 128, m * 128:(m + 1) * 128]
    hwT = hwn.reshape(NVV, H).T                    # [H, NVV]
    hw_tiles = np.concatenate([hwT[:128], hwT[128:]], axis=1)  # [128, 2*NVV]
    biasm = bb.reshape(8, 128).T.astype(np.float32).copy()     # [128, 8]
    headb = np.broadcast_to(head_b.reshape(NVV)[None, :],
                            (128, NVV)).astype(np.float32).copy()
    ident = np.eye(128, dtype=BF16)
    return {
        "wihT": np.ascontiguousarray(wihT),
        "whhT": np.ascontiguousarray(whh_tiles.astype(BF16)),
        "headwT": np.ascontiguousarray(hw_tiles.astype(BF16)),
        "biasm": biasm,
        "headb": headb,
        "ident": ident,
        "zeros16": np.zeros((128, 16), np.float32),
    }


def _make_consts(nonce):
    """Per-call consts: 0.5, 4.0, 2^23, nonce (stale-readback canary)."""
    c = np.broadcast_to(
        np.array([0.5, 4.0, 8388608.0, nonce], np.float32)[None, :],
        (NC * 128, 4))
    return np.ascontiguousarray(c)


_state = None


def _weights_key(ws):
    crc = 0
    for k in sorted(ws):
        crc = zlib.crc32(np.ascontiguousarray(ws[k]).view(np.uint8), crc)
    return crc


def _build_state(weight_arrays):
    """Build the Bass module once, jit the shard_map executable once, and
    upload weights to the 8 cores once."""
    bass2jax.install_neuronx_cc_hook()
    nc = build_nc()
    assert nc.dbg_addr is None
    partition_name = (nc.partition_id_tensor.name
                      if nc.partition_id_tensor else None)

    in_names, out_names, out_avals = [], [], []
    for alloc in nc.m.functions[0].allocations:
        if not isinstance(alloc, mybir.MemoryLocationSet):
            continue
        name = alloc.memorylocations[0].name
        if alloc.kind == "ExternalInput":
            if name != partition_name:
                in_names.append(name)
        elif alloc.kind == "ExternalOutput":
            out_names.append(name)
            out_avals.append(jax.core.ShapedArray(
                tuple(alloc.tensor_shape), mybir.dt.np(alloc.dtype)))
    n_params = len(in_names)
    all_names = list(in_names) + list(out_names)
    if partition_name is not None:
        all_names.append(partition_name)
    all_names = tuple(all_names)

    def _body(*args):
        operands = list(args)
        if partition_name is not None:
            operands.append(bass2jax.partition_id_tensor())
        outs = bass2jax._bass_exec_p.bind(
            *operands,
            out_avals=tuple(out_avals),
            in_names=all_names,
            out_names=tuple(out_names),
            lowering_input_output_aliases=(),
            sim_require_finite=True,
            sim_require_nnan=True,
            nc=nc,
        )
        return tuple(outs)

    devices = jax.devices()[:NC]
    mesh = Mesh(np.asarray(devices), ("core",))
    n_args = n_params + len(out_names)
    fn = jax.jit(
        shard_map(_body, mesh=mesh,
                  in_specs=(PartitionSpec("core"),) * n_args,
                  out_specs=(PartitionSpec("core"),) * len(out_names),
                  check_rep=False),
        donate_argnums=tuple(range(n_params, n_args)),
        keep_unused=True,
    )

    sharding = NamedSharding(mesh, PartitionSpec("core"))

    def _rowsum(a):
        return jnp.sum(a.astype(jnp.float32), axis=tuple(range(1, a.ndim)))

    rowsum_fn = jax.jit(_rowsum, out_shardings=sharding)
    wdev = {}
    for name, w in weight_arrays.items():
        g = np.ascontiguousarray(np.broadcast_to(w[None], (NC, *w.shape))
                                 .reshape(NC * w.shape[0], *w.shape[1:]))
        exp_sum = g.astype(np.float64).sum(
            axis=tuple(range(1, g.ndim))).astype(np.float32)
        for attempt in range(4):
            arr = jax.device_put(g, sharding)
            got = np.asarray(rowsum_fn(arr))
            if np.abs(got - exp_sum).max() < 0.1:
                break
        wdev[name] = arr

    # output seeds for donation: created on-device once; afterwards the
    # previous call's output buffers are donated (kernel writes every elem).
    seed_shapes = [((NC * av.shape[0], *av.shape[1:]), av.dtype)
                   for av in out_avals]
    seed_fn = jax.jit(
        lambda: tuple(jnp.zeros(s, d) for s, d in seed_shapes),
        out_shardings=(sharding,) * len(seed_shapes))
    seeds = list(seed_fn())
    for s in seeds:
        s.block_until_ready()

    st = {
        "fn": fn,
        "in_names": in_names,
        "out_names": out_names,
        "wdev": wdev,
        "seeds": seeds,
        "sharding": sharding,
    }

    # Two throwaway warm-up execs (zeros x): the first executions after NEFF
    # load have been observed to return partially-stale output DRAM (the
    # scales DMA lands incompletely); later executions are stable.  The
    # per-call nonce check in kernel() catches any residual staleness.
    xz = jax.jit(lambda: jnp.zeros((NC * 128, TOK), jnp.bfloat16),
                 out_shardings=sharding)()
    for w in range(2):
        cz = jax.device_put(_make_consts(-float(w + 1)), sharding)
        warm_args = [xz if n == "xT" else cz if n == "consts" else wdev[n]
                     for n in in_names]
        warm_args.extend(st["seeds"])
        st["seeds"] = list(fn(*warm_args))
        for s in st["seeds"]:
            s.block_until_ready()
    return st


def _put_x(x, sharding):
    """x [B,T,D] f32 -> device-resident global xT [NC*128, TOK] bf16 with
    rows [c*128+p], cols [t*BC+b] = x[c*BC+b, t, p].  Per-core prep overlaps
    the per-device uploads."""
    x = np.ascontiguousarray(x, np.float32)
    devices = jax.devices()[:NC]
    arrs = [None] * NC

    def one(c):
        xb = _to_bf16(x[c * BC:(c + 1) * BC])      # [BC, T, D] bf16
        xc = np.ascontiguousarray(
            xb.transpose(2, 1, 0).reshape(D, TOK))
        arrs[c] = jax.device_put(xc, devices[c])

    with ThreadPoolExecutor(NC) as ex:
        list(ex.map(one, range(NC)))
    return jax.make_array_from_single_device_arrays(
        (NC * D, TOK), sharding, arrs)


def kernel(x, W_ih, W_hh, b_ih, b_hh, head_w, head_b):
    global _state
    ws = _prep_weights(np.asarray(W_ih), np.asarray(W_hh), np.asarray(b_ih),
                       np.asarray(b_hh), np.asarray(head_w), np.asarray(head_b))
    wkey = _weights_key(ws)
    if _state is None or _state["wkey"] != wkey:
        st = _build_state(ws)
        st["wkey"] = wkey
        _state = st
    st = _state

    x_np = np.asarray(x)
    # expected per-(core,d) sums of x — verified against the device-side
    # reduction of the uploaded xT (the 16 MB upload has been observed to
    # land partially on cold processes)
    xsum_exp = np.asarray(x_np, np.float32).reshape(NC, BC, T, D).sum(
        axis=(1, 2)).reshape(NC * D)

    NT3 = TOK // 128
    full = np.empty((NV, B, T, V), np.float32)
    xdev = None
    with ThreadPoolExecutor(6) as ex:
        for attempt in range(5):
            if xdev is None:
                xdev = _put_x(x_np, st["sharding"])
            st["nonce"] = st.get("nonce", 0) + 1
            nonce = float(st["nonce"])
            cdev = jax.device_put(_make_consts(nonce), st["sharding"])
            args = [xdev if n == "xT" else cdev if n == "consts"
                    else st["wdev"][n] for n in st["in_names"]]
            args.extend(st["seeds"])
            outs = st["fn"](*args)
            st["seeds"] = list(outs)   # donate these buffers next call
            out_by_name = dict(zip(st["out_names"], outs))
            lg, sc = out_by_name["logits"], out_by_name["scales"]
            # per-shard .data fetches do not synchronize with the in-flight
            # donated-alias execution — block first
            lg.block_until_ready()
            sc.block_until_ready()
            # fetch scales and logits shards concurrently
            sc_fut = ex.submit(lambda a=sc: np.asarray(a))
            lg_shards = sorted(lg.addressable_shards,
                               key=lambda s: s.index[0].start or 0)
            q_futs = [ex.submit(lambda s_=s_: np.asarray(s_.data))
                      for s_ in lg_shards]
            sc_np = sc_fut.result()
            # nonce column proves the scales DMA of THIS call landed; xsum
            # column proves the x upload arrived intact
            nonce_ok = ((sc_np[:, NT3] == nonce).all()
                        and np.isfinite(sc_np).all())
            xsum_ok = np.abs(sc_np[:, NT3 + 1] - xsum_exp).max() < 2.0
            qs = [f.result() for f in q_futs]
            if nonce_ok and xsum_ok:
                break
            if not xsum_ok:
                xdev = None            # force a fresh upload

        def _dequant(c_q):
            c, q = c_q                             # q: [NV, BC, T, V] int8
            s_core = sc_np[c * 128:(c + 1) * 128, :NT3]   # applied scale
            # token (tk*128 + p) = t*BC + b  ->  [T, BC] inverse scale grid
            inv = (1.0 / s_core.T.reshape(T, BC)).astype(np.float32)
            np.multiply(q, inv.T[None, :, :, None],
                        out=full[:, c * BC:(c + 1) * BC], casting="unsafe")

        list(ex.map(_dequant, enumerate(qs)))
    return (full[0], full[1], full[2])


# revision 44
# speedup vs baseline: 1.1824x; 1.1824x over previous
"""Trainium2 Bass kernel for nn_LSTMHarmonizer — V3 (transfer-optimized).

The on-device kernel (per core: gx GEMM -> 1024-step LSTM scan -> 3-head
GEMM) runs in ~1ms; the axon tunnel to the devices moves ~40-60 MB/s, so
wall time is dominated by host<->device bytes.  V3 therefore:
  - caches the jitted shard_map executable across kernel() calls (the
    stock run_bass_kernel_spmd re-traces + re-lowers every call),
  - keeps weights device-resident (uploaded once, content-checked),
  - emits logits in f16 (50 MB down instead of 100), upcast on host,
  - donates the PREVIOUS call's device-resident output as the next
    call's output seed (kernel writes every element, so no 100 MB zeros
    upload per call),
  - fetches output shards with a small thread pool (~1.4x).

Scan math (unchanged from V2): gate tiles m = f0 g0 i0 o0 f1 g1 i1 o1;
c' tracks 2c with g rows pre-scaled 2x (tanh via 2*sigma(2x)-1); h is
stored as h/2 in bf16 with W_hh and head_w pre-scaled 2x.
"""

import contextlib
import zlib
from concurrent.futures import ThreadPoolExecutor

import numpy as np
import ml_dtypes

import jax
import jax.numpy as jnp
from jax.sharding import Mesh, PartitionSpec, NamedSharding
from jax.experimental.shard_map import shard_map

import concourse.bass as bass
import concourse.mybir as mybir
from concourse import bass2jax

BF16 = ml_dtypes.bfloat16

B, T, D, H, V, NV = 64, 1024, 128, 256, 128, 3
G4 = 4 * H            # 1024
NC = 8                # cores
BC = B // NC          # 8 sequences per core
NVV = NV * V          # 384
TOK = BC * T          # 8192 tokens per core

# k1-phase / o-phase tile orders (m indices)
CJ = [0, 1, 2, 3, 4, 5]       # c-gates: f0 g0 i0, f1 g1 i1
OG = [(6, 0), (7, 0), (6, 1), (7, 1)]  # o-gates (m, k)


def build_nc(Tc=T):
    TOKc = BC * Tc
    NT3 = TOKc // 128          # phase-3 token tiles
    NCH = TOKc // 512          # phase-1 token chunks
    f32 = mybir.dt.float32
    bf16 = mybir.dt.bfloat16
    f16 = mybir.dt.float16

    i8 = mybir.dt.int8

    nc = bass.Bass()
    xT_d = nc.declare_dram_parameter("xT", [128, TOKc], bf16, isOutput=False)
    wih_d = nc.declare_dram_parameter("wihT", [128, G4], bf16, isOutput=False)
    whh_d = nc.declare_dram_parameter("whhT", [128, 16 * 128], bf16, isOutput=False)
    hw_d = nc.declare_dram_parameter("headwT", [128, 2 * NVV], bf16, isOutput=False)
    bias_d = nc.declare_dram_parameter("biasm", [128, 8], f32, isOutput=False)
    hb_d = nc.declare_dram_parameter("headb", [128, NVV], f32, isOutput=False)
    id_d = nc.declare_dram_parameter("ident", [128, 128], bf16, isOutput=False)
    z_d = nc.declare_dram_parameter("zeros16", [128, 16], f32, isOutput=False)
    cst_d = nc.declare_dram_parameter("consts", [128, 4], f32, isOutput=False)
    lg_d = nc.declare_dram_parameter("logits", [NV, BC, Tc, V], i8, isOutput=True)
    # last column carries the per-call nonce (stale-readback canary)
    sc_d = nc.declare_dram_parameter("scales", [128, TOKc // 128 + 2], f32,
                                     isOutput=True)

    ctx = contextlib.ExitStack()
    with ctx:
        sb = lambda name, shape, dt: ctx.enter_context(
            nc.sbuf_tensor(name, shape, dt))
        ps = lambda name, shape: ctx.enter_context(
            nc.psum_tensor(name, shape, f32))
        sem = lambda name: ctx.enter_context(nc.semaphore(name))

        xT = sb("xT_s", [128, TOKc], bf16)
        wih = sb("wih_s", [128, G4], bf16)
        whh = sb("whh_s", [128, 16 * 128], bf16)
        hw = sb("hw_s", [128, 2 * NVV], bf16)
        biasm = sb("biasm_s", [128, 8], f32)
        headb = sb("headb_s", [128, NVV], f32)
        ident = sb("ident_s", [128, 128], bf16)
        zeros16 = sb("zeros16_s", [128, 16], f32)
        cst = sb("cst_s", [128, 4], f32)   # 0.5, 4.0, 2^23, nonce
        gx = sb("gx_s", [128, 8 * TOKc], bf16)      # (t, m, b)
        hh = sb("hh_s", [128, 2 * TOKc], bf16)      # (j, t, b), holds h/2
        sall = sb("sall_s", [128, 64], f32)        # sigma of gates (m, b)
        scb = sb("scb_s", [128, 16], f32)          # sigma(c') per (j, b)
        vu = sb("vu_s", [128, 32], f32)            # v (0:16), u (16:32)
        cps = sb("cps_s", [128, 16], f32)          # c' (=2c): j0 0:8, j1 8:16
        # phase-3 int8 quantization buffers (partition = token-in-tile)
        scl = sb("scl_s", [128, NT3], f32)         # per-token absmax
        sclm = sb("sclm_s", [128, NT3], f32)       # clamped absmax
        sinv = sb("sinv_s", [128, NT3], f32)       # 1/absmax
        sqs = sb("sqs_s", [128, NT3 + 2], f32)     # 126/absmax|nonce|xsum
        tmpb = sb("tmpb_s", [128, 2 * NVV], f32)   # s*x + 2^23, 2 slots
        qout = sb("qout_s", [128, 4 * NVV], i8)    # int8 evict slots

        # 6 full banks: [0..3] phases 1/3; scan uses [t%2], [2+t%2], [4+t%2]
        ps_big = [ps(f"psb{i}", [128, 512]) for i in range(6)]

        dma_in = sem("dma_in")
        mm1 = sem("mm1")
        ev1a = sem("ev1a")
        ev1d = sem("ev1d")
        s_mm = sem("s_mm")     # 3/step: c-j0, c-j1, o
        s_sig = sem("s_sig")   # 5/step: A1 A3 A2' sc0 sc1
        s_cp = sem("s_cp")     # init=2, then 2/step: c'0 (2t+3), c'1 (2t+4)
        s_vu = sem("s_vu")     # 2/step: u0 (2t+1), u1 (2t+2)
        s_h = sem("s_h")       # 2/step: h0, h1
        mm3 = sem("mm3")
        s_pa = sem("s_pa")     # 1/tile: pass A (psum consumed)
        s_sc = sem("s_sc")     # 1: applied scales ready
        s_t2 = sem("s_t2")     # 1/tile: pass B scalar (s*x + 2^23)
        ev3 = sem("ev3")       # 1/tile: pass B vector (int8 slot ready)
        dma_out = sem("dma_out")

        ALU = mybir.AluOpType
        AF = mybir.ActivationFunctionType

        # Semaphores are NOT cleared on allocation, and nothing clears them
        # between executions of a loaded NEFF (the target_bir_lowering
        # preamble that would is not emitted on the bass_exec path).  On
        # re-execution every wait_ge would pass against stale counts and all
        # engines would free-run.  Clear our sems and barrier before use.
        all_sems = [dma_in, mm1, ev1a, ev1d, s_mm, s_sig, s_cp, s_vu, s_h,
                    mm3, s_pa, s_sc, s_t2, ev3, dma_out]
        for rng in bass.compact_to_ranges([s.num for s in all_sems]):
            nc.gpsimd.dma_reset(rng)
            nc.gpsimd.sem_clear(rng)
        nc.all_engine_barrier()

        gx_v = gx[:].rearrange("p (t m b) -> p t m b", t=Tc, m=8, b=BC)

        def gx_evict_view(m, ch):
            return gx_v[:, ch * 64:(ch + 1) * 64, m, :]

        def hh_ap(j, t):
            off = j * TOKc + t * BC
            return hh[:, off:off + BC]

        def bc0(t):
            return ps_big[t % 2]
        def bc1(t):
            return ps_big[2 + t % 2]
        def bo(t):
            return ps_big[4 + t % 2]

        def wt(k, m):
            return whh[:, (k * 8 + m) * 128:(k * 8 + m + 1) * 128]

        with nc.Block() as block:

            @block.sync
            def _(sync):
                sync.dma_start(out=xT[:], in_=xT_d[:]).then_inc(dma_in, 16)
                sync.dma_start(out=wih[:], in_=wih_d[:]).then_inc(dma_in, 16)
                sync.dma_start(out=whh[:], in_=whh_d[:]).then_inc(dma_in, 16)
                sync.dma_start(out=hw[:], in_=hw_d[:]).then_inc(dma_in, 16)
                sync.dma_start(out=biasm[:], in_=bias_d[:]).then_inc(dma_in, 16)
                sync.dma_start(out=headb[:], in_=hb_d[:]).then_inc(dma_in, 16)
                sync.dma_start(out=ident[:], in_=id_d[:]).then_inc(dma_in, 16)
                sync.dma_start(out=zeros16[:], in_=z_d[:]).then_inc(dma_in, 16)
                sync.dma_start(out=cst[:], in_=cst_d[:]).then_inc(dma_in, 16)
                # phase 3 output DMAs
                for tk in range(NT3):
                    sync.wait_ge(ev3, tk + 1)
                    for n in range(NV):
                        dview = lg_d[n, :, tk * 16:(tk + 1) * 16, :].rearrange(
                            "b t v -> t b v")
                        slot = qout[:, (tk % 4) * NVV + n * V:
                                    (tk % 4) * NVV + (n + 1) * V]
                        sync.dma_start(out=dview, in_=slot).then_inc(dma_out, 16)
                sync.wait_ge(s_sc, 1)
                sync.dma_start(out=sc_d[:], in_=sqs[:]).then_inc(dma_out, 16)
                sync.wait_ge(dma_out, 48 * NT3 + 16)

            @block.tensor
            def _(tensor):
                tensor.wait_ge(dma_in, 144)
                # ---- phase 1: gx GEMM ----
                for m in range(8):
                    for ch in range(NCH):
                        idx = m * NCH + ch
                        if idx >= 4:
                            j = idx - 4
                            if j % 2 == 0:
                                tensor.wait_ge(ev1a, j // 2 + 1)
                            else:
                                tensor.wait_ge(ev1d, (j + 1) // 2)
                        tensor.matmul(
                            ps_big[idx % 4][:, :512],
                            lhsT=wih[:, m * 128:(m + 1) * 128],
                            rhs=xT[:, ch * 512:(ch + 1) * 512],
                            start=True, stop=True,
                        ).then_inc(mm1, 1)
                # ---- phase 2: scan ----
                tensor.wait_ge(ev1a, 4 * NCH)
                tensor.wait_ge(ev1d, 4 * NCH)
                def gout(t, m):
                    # psum slot for gate tile m of step t
                    if m < 3:
                        return bc0(t)[:, m * 8:(m + 1) * 8]
                    if m < 6:
                        return bc1(t)[:, (m - 3) * 8:(m - 2) * 8]
                    return bo(t)[:, (m - 6) * 8:(m - 5) * 8]

                gxs = lambda t, a, b2: gx[:, t * 64 + a:t * 64 + b2]
                for t in range(Tc):
                    if t >= 2:
                        tensor.wait_ge(s_sig, 5 * (t - 2) + 3)
                    tensor.matmul(
                        bc0(t)[:, 0:24], lhsT=ident[:], rhs=gxs(t, 0, 24),
                        start=True, stop=(t == 0), skip_group_check=True)
                    tensor.matmul(
                        bc1(t)[:, 0:24], lhsT=ident[:], rhs=gxs(t, 24, 48),
                        start=True, stop=(t == 0), skip_group_check=True)
                    ins0 = tensor.matmul(
                        bo(t)[:, 0:16], lhsT=ident[:], rhs=gxs(t, 48, 64),
                        start=True, stop=(t == 0), skip_group_check=True)
                    if t == 0:
                        ins0.then_inc(s_mm, 1)
                        continue
                    # c-gates k0 phase
                    tensor.wait_ge(s_h, 2 * t - 1)
                    for m in CJ:
                        tensor.matmul(
                            gout(t, m), lhsT=wt(0, m),
                            rhs=hh_ap(0, t - 1), start=False, stop=False,
                            skip_group_check=True)
                    # c-gates k1 phase
                    tensor.wait_ge(s_h, 2 * t)
                    for i, m in enumerate(CJ):
                        ins = tensor.matmul(
                            gout(t, m), lhsT=wt(1, m),
                            rhs=hh_ap(1, t - 1), start=False, stop=(i in (2, 5)),
                            skip_group_check=True)
                        if i == 2:
                            ins.then_inc(s_mm, 1)     # c-j0 done: 3t-1
                        elif i == 5:
                            ins.then_inc(s_mm, 1)     # c-j1 done: 3t
                    # o-gates
                    for i, (m, k) in enumerate(OG):
                        ins = tensor.matmul(
                            gout(t, m), lhsT=wt(k, m),
                            rhs=hh_ap(k, t - 1), start=False,
                            stop=(i == 3), skip_group_check=True)
                        if i == 3:
                            ins.then_inc(s_mm, 1)     # o done: 3t+1
                # ---- phase 3: heads ----
                tensor.wait_ge(s_h, 2 * Tc)
                for tk in range(NT3):
                    if tk >= 4:
                        tensor.wait_ge(s_pa, tk - 3)
                    tensor.matmul(
                        ps_big[tk % 4][:, :NVV],
                        lhsT=hh[:, tk * 128:tk * 128 + 128],
                        rhs=hw[:, :NVV], start=True, stop=False,
                        skip_group_check=True,
                    )
                    tensor.matmul(
                        ps_big[tk % 4][:, :NVV],
                        lhsT=hh[:, TOKc + tk * 128:TOKc + tk * 128 + 128],
                        rhs=hw[:, NVV:2 * NVV], start=False, stop=True,
                        skip_group_check=True,
                    ).then_inc(mm3, 1)

            @block.scalar
            def _(scalar):
                scalar.wait_ge(dma_in, 144)
                # phase-1 evicts: even tiles
                for idx in range(0, 8 * NCH, 2):
                    m, ch = idx // NCH, idx % NCH
                    scalar.wait_ge(mm1, idx + 1)
                    scalar.activation(
                        out=gx_evict_view(m, ch),
                        in_=ps_big[idx % 4][:, :512].rearrange(
                            "p (t b) -> p t b", t=64, b=BC),
                        func=AF.Identity, bias=biasm[:, m:m + 1],
                    ).then_inc(ev1a, 1)
                # scan
                for t in range(Tc):
                    scalar.wait_ge(s_mm, 3 * t - 1 if t else 1)
                    scalar.activation(out=sall[:, 0:24], in_=bc0(t)[:, 0:24],
                                      func=AF.Sigmoid).then_inc(s_sig, 1)
                    scalar.wait_ge(s_mm, 3 * t if t else 1)
                    scalar.activation(out=sall[:, 24:48], in_=bc1(t)[:, 0:24],
                                      func=AF.Sigmoid).then_inc(s_sig, 1)
                    scalar.wait_ge(s_mm, 3 * t + 1)
                    scalar.activation(out=sall[:, 48:64], in_=bo(t)[:, 0:16],
                                      func=AF.Sigmoid).then_inc(s_sig, 1)
                    scalar.wait_ge(s_cp, 2 * t + 2)
                    scalar.activation(out=scb[:, 0:8], in_=cps[:, 0:8],
                                      func=AF.Sigmoid).then_inc(s_sig, 1)
                    scalar.wait_ge(s_cp, 2 * t + 3)
                    scalar.activation(out=scb[:, 8:16], in_=cps[:, 8:16],
                                      func=AF.Sigmoid).then_inc(s_sig, 1)
                # phase-3 pass B (1/2): tmp = s*x + 2^23  (rounds to int grid)
                scalar.wait_ge(s_sc, 1)
                for tk in range(NT3):
                    if tk >= 2:
                        scalar.wait_ge(ev3, tk - 1)
                    scalar.activation(
                        out=tmpb[:, (tk % 2) * NVV:(tk % 2 + 1) * NVV],
                        in_=gx[:, tk * NVV:(tk + 1) * NVV],
                        func=AF.Identity,
                        scale=sqs[:, tk:tk + 1], bias=cst[:, 2:3],
                    ).then_inc(s_t2, 1)

            @block.vector
            def _(vector):
                vector.wait_ge(dma_in, 144)
                # x-upload checksum: per-partition sum of xT (host verifies)
                vector.tensor_reduce(
                    out=sqs[:, NT3 + 1:NT3 + 2], in_=xT[:],
                    axis=mybir.AxisListType.X, op=ALU.add)
                # phase-1 evicts: odd tiles
                for idx in range(1, 8 * NCH, 2):
                    m, ch = idx // NCH, idx % NCH
                    vector.wait_ge(mm1, idx + 1)
                    vector.tensor_scalar_add(
                        gx_evict_view(m, ch),
                        ps_big[idx % 4][:, :512].rearrange(
                            "p (t b) -> p t b", t=64, b=BC),
                        biasm[:, m:m + 1],
                    ).then_inc(ev1d, 1)
                # init c' = 0
                vector.tensor_copy(cps[:], zeros16[:]).then_inc(s_cp, 1)
                # scan: sall cols (m,b): f0 g0 i0 o0 f1 g1 i1 o1
                for t in range(Tc):
                    for j in range(2):
                        o = 24 * j
                        vector.wait_ge(s_sig, 5 * t + 1 + j)
                        vector.wait_ge(s_cp, max(1, 2 * t + j))
                        # v_j = sig(f_j) * c'_old_j
                        vector.tensor_tensor(
                            out=vu[:, j * 8:j * 8 + 8], in0=sall[:, o:o + 8],
                            in1=cps[:, j * 8:(j + 1) * 8], op=ALU.mult)
                        # u_j = (sig(2g_j) - 0.5) * sig(i_j)
                        vector.scalar_tensor_tensor(
                            out=vu[:, 16 + j * 8:24 + j * 8],
                            in0=sall[:, o + 8:o + 16], scalar=cst[:, 0:1],
                            in1=sall[:, o + 16:o + 24],
                            op0=ALU.subtract, op1=ALU.mult).then_inc(s_vu, 1)
                    for j in range(2):
                        # c'_j = 4*u_j + v_j
                        vector.wait_ge(s_vu, 2 * t + 1 + j)
                        vector.scalar_tensor_tensor(
                            out=cps[:, j * 8:(j + 1) * 8],
                            in0=vu[:, 16 + j * 8:24 + j * 8], scalar=cst[:, 1:2],
                            in1=vu[:, j * 8:j * 8 + 8],
                            op0=ALU.mult, op1=ALU.add).then_inc(s_cp, 1)
                    for j in range(2):
                        # h_j/2 = (sc_j - 0.5) * sig(o_j)
                        vector.wait_ge(s_sig, 5 * t + 4 + j)
                        vector.scalar_tensor_tensor(
                            out=hh_ap(j, t), in0=scb[:, j * 8:(j + 1) * 8],
                            scalar=cst[:, 0:1],
                            in1=sall[:, 48 + 8 * j:56 + 8 * j],
                            op0=ALU.subtract, op1=ALU.mult).then_inc(s_h, 1)
                # phase-3 pass A: bias add (bf16, into dead gx region) +
                # per-token absmax of the stored values.  The reduce runs one
                # tile BEHIND the add: a same-engine read immediately after a
                # large SBUF write has been observed to catch stale bytes
                # (write-drain hazard), so every RAW pair below is separated
                # by at least one intervening instruction.
                def lgb(tk):
                    return gx[:, tk * NVV:(tk + 1) * NVV]

                for tk in range(NT3):
                    vector.wait_ge(mm3, tk + 1)
                    vector.tensor_tensor(
                        out=lgb(tk), in0=ps_big[tk % 4][:, :NVV], in1=headb[:],
                        op=ALU.add).then_inc(s_pa, 1)
                    if tk >= 1:
                        vector.tensor_reduce(
                            out=scl[:, tk - 1:tk], in_=lgb(tk - 1),
                            axis=mybir.AxisListType.X, op=ALU.max,
                            apply_absolute_value=True)
                vector.tensor_copy(sqs[:, NT3:NT3 + 1], cst[:, 3:4])  # nonce
                vector.tensor_reduce(
                    out=scl[:, NT3 - 1:NT3], in_=lgb(NT3 - 1),
                    axis=mybir.AxisListType.X, op=ALU.max,
                    apply_absolute_value=True)
                # applied scale s = 126/max(|x|, eps); shipped verbatim.
                # memsets of dead buffers space the RAW chain.
                vector.memset(vu[:, 0:8], 0.0)
                vector.tensor_scalar_max(sclm[:], scl[:], 1e-20)
                vector.memset(vu[:, 8:16], 0.0)
                vector.reciprocal(sinv[:], sclm[:])
                vector.memset(vu[:, 16:24], 0.0)
                vector.tensor_scalar_mul(sqs[:, :NT3], sinv[:], 126.0)
                vector.memset(vu[:, 24:32], 0.0)
                vector.tensor_copy(scb[:, 0:8], zeros16[:, 0:8]).then_inc(
                    s_sc, 1)
                # phase-3 pass B (2/2): int8 = (tmp - 2^23), exact integer
                for tk in range(NT3):
                    vector.wait_ge(s_t2, tk + 1)
                    if tk >= 4:
                        vector.wait_ge(dma_out, 48 * (tk - 3))
                    vector.tensor_scalar(
                        out=qout[:, (tk % 4) * NVV:(tk % 4 + 1) * NVV],
                        in0=tmpb[:, (tk % 2) * NVV:(tk % 2 + 1) * NVV],
                        scalar1=cst[:, 2:3], scalar2=None,
                        op0=ALU.subtract).then_inc(ev3, 1)

    return nc


def _to_bf16(a):
    """Fast f32 -> bf16 with round-to-nearest-even (finite inputs)."""
    u = np.ascontiguousarray(a, np.float32).view(np.uint32)
    r = ((u >> 16) & 1) + np.uint32(0x7FFF)
    return ((u + r) >> 16).astype(np.uint16).view(BF16)


def _prep_weights(W_ih, W_hh, b_ih, b_hh, head_w, head_b):
    # gate order (i,f,g,o) -> m-tiles (f0 g0 i0 o0 f1 g1 i1 o1)
    a = np.arange
    perm = np.concatenate([
        a(256, 384), a(512, 640), a(0, 128),
        a(384, 512), a(640, 768), a(128, 256),
        a(768, 896), a(896, 1024)])
    g_rows = np.concatenate([a(128, 256), a(512, 640)])  # g0, g1 in new order
    wi = W_ih[perm].astype(np.float64).copy()
    wh = W_hh[perm].astype(np.float64).copy()
    bb = (b_ih + b_hh)[perm].astype(np.float64).copy()
    # tanh fold: g rows x2 everywhere; h stored as h/2: W_hh x2, head_w x2
    wi[g_rows] *= 2.0
    bb[g_rows] *= 2.0
    wh *= 2.0
    wh[g_rows] *= 2.0
    hwn = 2.0 * head_w.astype(np.float64)

    wihT = wi.T.astype(BF16)                       # [D, G4]
    whhT_f = wh.T                                  # [H, G4]
    whh_tiles = np.zeros((128, 16 * 128), np.float64)
    for k in range(2):
        for m in range(8):
            whh_tiles[:, (k * 8 + m) * 128:(k * 8 + m + 1) * 128] = \
                whhT_f[k * 128:(k + 1) * 128, m * 128:(m + 1) * 128]
    hwT = hwn.reshape(NVV, H).T                    # [H, NVV]
    hw_tiles = np.concatenate([hwT[:128], hwT[128:]], axis=1)  # [128, 2*NVV]
    biasm = bb.reshape(8, 128).T.astype(np.float32).copy()     # [128, 8]
    headb = np.broadcast_to(head_b.reshape(NVV)[None, :],
                            (128, NVV)).astype(np.float32).copy()
    ident = np.eye(128, dtype=BF16)
    return {
        "wihT": np.ascontiguousarray(wihT),
        "whhT": np.ascontiguousarray(whh_tiles.astype(BF16)),
        "headwT": np.ascontiguousarray(hw_tiles.astype(BF16)),
        "biasm": biasm,
        "headb": headb,
        "ident": ident,
        "zeros16": np.zeros((128, 16), np.float32),
    }


def _make_consts(nonce):
    """Per-call consts: 0.5, 4.0, 2^23, nonce (stale-readback canary)."""
    c = np.broadcast_to(
        np.array([0.5, 4.0, 8388608.0, nonce], np.float32)[None, :],
        (128, 4))
    return np.ascontiguousarray(c)


_state = None


def _weights_key(ws):
    crc = 0
    for k in sorted(ws):
        crc = zlib.crc32(np.ascontiguousarray(ws[k]).view(np.uint8), crc)
    return crc


def _build_state(weight_arrays):
    """Build the Bass module once, jit the shard_map executable once, and
    upload weights to the 8 cores once."""
    bass2jax.install_neuronx_cc_hook()
    nc = build_nc()
    assert nc.dbg_addr is None
    partition_name = (nc.partition_id_tensor.name
                      if nc.partition_id_tensor else None)

    in_names, out_names, out_avals = [], [], []
    for alloc in nc.m.functions[0].allocations:
        if not isinstance(alloc, mybir.MemoryLocationSet):
            continue
        name = alloc.memorylocations[0].name
        if alloc.kind == "ExternalInput":
            if name != partition_name:
                in_names.append(name)
        elif alloc.kind == "ExternalOutput":
            out_names.append(name)
            out_avals.append(jax.core.ShapedArray(
                tuple(alloc.tensor_shape), mybir.dt.np(alloc.dtype)))
    n_params = len(in_names)
    all_names = list(in_names) + list(out_names)
    if partition_name is not None:
        all_names.append(partition_name)
    all_names = tuple(all_names)

    def _body(*args):
        operands = list(args)
        if partition_name is not None:
            operands.append(bass2jax.partition_id_tensor())
        outs = bass2jax._bass_exec_p.bind(
            *operands,
            out_avals=tuple(out_avals),
            in_names=all_names,
            out_names=tuple(out_names),
            lowering_input_output_aliases=(),
            sim_require_finite=True,
            sim_require_nnan=True,
            nc=nc,
        )
        return tuple(outs)

    devices = jax.devices()[:NC]
    n_args = n_params + len(out_names)
    fn1 = jax.jit(_body, donate_argnums=tuple(range(n_params, n_args)),
                  keep_unused=True)

    from jax.sharding import SingleDeviceSharding

    def _rowsum(a):
        return jnp.sum(a.astype(jnp.float32), axis=tuple(range(1, a.ndim)))

    rowsum_fn = jax.jit(_rowsum)
    wdev = {}          # name -> [per-core single-device arrays]
    for name, w in weight_arrays.items():
        exp_sum = w.astype(np.float64).sum(
            axis=tuple(range(1, w.ndim))).astype(np.float32)
        arrs = []
        for c in range(NC):
            for attempt in range(4):
                arr = jax.device_put(w, devices[c])
                got = np.asarray(rowsum_fn(arr))
                if np.abs(got - exp_sum).max() < 0.1:
                    break
            arrs.append(arr)
        wdev[name] = arrs

    # per-core output seeds for donation
    seed_shapes = [(tuple(av.shape), av.dtype) for av in out_avals]
    seeds = []
    for c in range(NC):
        zf = jax.jit(lambda: tuple(jnp.zeros(sh, d) for sh, d in seed_shapes),
                     out_shardings=SingleDeviceSharding(devices[c]))
        sl = list(zf())
        for s_ in sl:
            s_.block_until_ready()
        seeds.append(sl)

    st = {
        "fn": fn1,
        "in_names": in_names,
        "out_names": out_names,
        "wdev": wdev,
        "seeds": seeds,
        "devices": devices,
    }

    # Two throwaway warm-up execs per core (zeros x): cold first executions
    # have been observed to misbehave (sem-clear + write-drain hazards); the
    # per-call nonce/xsum checks catch any residual issue.
    for c in range(NC):
        xz = jax.device_put(np.zeros((128, TOK), BF16), devices[c])
        cz = jax.device_put(_make_consts(-1.0), devices[c])
        for w in range(2):
            warm_args = [xz if n == "xT" else cz if n == "consts"
                         else wdev[n][c] for n in in_names]
            warm_args.extend(st["seeds"][c])
            st["seeds"][c] = list(fn1(*warm_args))
    for c in range(NC):
        for s_ in st["seeds"][c]:
            s_.block_until_ready()
    return st


def _prep_x_core(x, c):
    """Core c slice of x -> [128, TOK] bf16; cols [t*BC+b] = x[c*BC+b,t,p]."""
    xb = _to_bf16(x[c * BC:(c + 1) * BC])          # [BC, T, D] bf16
    return np.ascontiguousarray(xb.transpose(2, 1, 0).reshape(D, TOK))


def kernel(x, W_ih, W_hh, b_ih, b_hh, head_w, head_b):
    global _state
    ws = _prep_weights(np.asarray(W_ih), np.asarray(W_hh), np.asarray(b_ih),
                       np.asarray(b_hh), np.asarray(head_w), np.asarray(head_b))
    wkey = _weights_key(ws)
    if _state is None or _state["wkey"] != wkey:
        st = _build_state(ws)
        st["wkey"] = wkey
        _state = st
    st = _state

    x_np = np.ascontiguousarray(np.asarray(x), np.float32)
    # expected per-(core,d) sums of x — verified against the device-side
    # reduction of the uploaded xT (cold uploads have been observed to land
    # partially)
    xsum_exp = x_np.reshape(NC, BC, T, D).sum(axis=(1, 2))   # [NC, D]

    NT3 = TOK // 128
    full = np.empty((NV, B, T, V), np.float32)
    st["nonce"] = st.get("nonce", 0) + 1
    nonce = float(st["nonce"])
    cst_np = _make_consts(nonce)

    def dispatch(c, xdev):
        cdev = jax.device_put(cst_np, st["devices"][c])
        args = [xdev if n == "xT" else cdev if n == "consts"
                else st["wdev"][n][c] for n in st["in_names"]]
        args.extend(st["seeds"][c])
        outs = st["fn"](*args)
        st["seeds"][c] = list(outs)
        return dict(zip(st["out_names"], outs))

    def fetch(c, ob):
        lg_c, sc_c = ob["logits"], ob["scales"]
        lg_c.block_until_ready()
        sc_c.block_until_ready()
        sn = np.asarray(sc_c)                      # [128, NT3+2]
        nonce_ok = (sn[:, NT3] == nonce).all() and np.isfinite(sn).all()
        xsum_ok = np.abs(sn[:, NT3 + 1] - xsum_exp[c]).max() < 2.0
        if not (nonce_ok and xsum_ok):
            return False, xsum_ok
        q = np.asarray(lg_c)                       # [NV, BC, T, V] int8
        inv = (1.0 / sn[:, :NT3].T.reshape(T, BC)).astype(np.float32)
        np.multiply(q, inv.T[None, :, :, None],
                    out=full[:, c * BC:(c + 1) * BC], casting="unsafe")
        return True, True

    with ThreadPoolExecutor(NC + 2) as ex:
        # wavefront: issue core c's upload+dispatch, then its fetch task;
        # early cores stream results back (duplex) while late cores upload
        futs = []
        for c in range(NC):
            xdev = jax.device_put(_prep_x_core(x_np, c), st["devices"][c])
            ob = dispatch(c, xdev)
            futs.append(ex.submit(fetch, c, ob))
        for c, fut in enumerate(futs):
            ok, xsum_ok = fut.result()
            for attempt in range(4):
                if ok:
                    break
                xdev = jax.device_put(_prep_x_core(x_np, c),
                                      st["devices"][c])
                ob = dispatch(c, xdev)
                ok, xsum_ok = fetch(c, ob)
    return (full[0], full[1], full[2])
